# revision 37
# baseline (speedup 1.0000x reference)
"""Trainium2 Bass kernel for nn_BasicBlock (per-sample dynamic 3x3 convs +
sync-BN + residual ReLU), data-parallel over batch on 8 NeuronCores.

Reference semantics (B=16, C=64, H=W=128):
    out = relu(bn2(conv2(relu(bn1(conv1(x, f1))), f2)) + x)
with training-mode BN over full-batch (N,H,W) statistics.

Sharding: 2 samples per core. Per-sample convs become block-diagonal
128x128 matmuls (partitions 0-63 = sample A channels, 64-127 = sample B).
BN batch statistics are made exact via a tiny AllReduce of per-channel
(mean, var, mean^2) over the 16 (sample, core) groups.

conv1 runs as 4-quadrant PE-tiled matmuls (tile_position via
base_partition): per round, two taps x two samples stream concurrently
through the four 64x64 quadrant groups of the PE array (100% useful
weights), using host-provided partition-swapped copies of the padded
image and conv1 weights; even/odd tap chains accumulate in two PSUM
banks summed by DVE at evacuation (9 taps -> 5 rounds, 1.8x fewer
streaming cycles). conv2 stays block-diagonal 9-tap ([128,512] PSUM
accumulation). Matmul inputs are bf16; accumulation is fp32. Raw conv
outputs are staged bf16 for BN stats, normalization math is fp32.

v2 critical-path work:
- warmup AllReduce doorbell first-thing on gpsimd, sized like the real
  stats AR, so the one-time CC entry barrier + first-AR costs burn during
  conv1 instead of in front of the BN1 AllReduce.
- stats DMA + collective doorbell both on gpsimd (queue-local ordering,
  no cross-engine semaphore hop).
- norm_pad borders zeroed by DVE memsets, not element-strided DMAs
  (the old column DMAs emitted ~33k 2-byte packets).
- conv1 weights DMA'd separately ahead of conv2 weights; x chunks
  front-loaded so the first matmul issues as early as possible.
- dummy matmuls between conv1 and conv2 keep the PE HAM clock at 2.4GHz
  across the AllReduce stall (otherwise conv2 starts at 1.2GHz).
"""
import numpy as np

import concourse.bass as bass
import concourse.mybir as mybir
import concourse.tile as tile
from concourse import bacc
from concourse.bass_utils import run_bass_kernel_spmd
N_CORES = 8
B, C, H, W = 16, 64, 128, 128
SPC = B // N_CORES            # samples per core (2)
HP, WP = H + 2, W + 2         # padded image
TR = 4                        # image rows per spatial tile
NT = H // TR                  # 32 tiles
N = TR * W                    # 512 moving elements per matmul
NGROUPS = B                   # 16 (sample, core) stat groups of H*W each
BN_EPS = 1e-5
N_DUMMY = 110                 # PE warm-keeper matmuls during the BN1 AR stall

F32 = mybir.dt.float32
F32R = mybir.dt.float32r
BF16 = mybir.dt.bfloat16
USE_BF16_MM = True          # bf16 matmul inputs (fast weight load) vs f32r
MMDT = BF16 if USE_BF16_MM else F32R
AF = mybir.ActivationFunctionType
ALU = mybir.AluOpType

_CACHE = {}


def _build():
    nc = bacc.Bacc("TRN2", target_bir_lowering=False, debug=False,
                   num_devices=N_CORES)
    xp_ext = nc.dram_tensor("xp", [128, HP, WP], MMDT, kind="ExternalInput").ap()
    # partition-swapped copy of xp (sample B on 0:64, A on 64:128) for the
    # 4-quadrant conv1 packing
    xs_ext = nc.dram_tensor("xs", [128, HP, WP], MMDT, kind="ExternalInput").ap()
    w_ext = nc.dram_tensor("w", [128, 2, 9, 128], MMDT, kind="ExternalInput").ap()
    # partition-swapped weights, both convs (for quadrants fed by swapped images)
    ws_ext = nc.dram_tensor("ws", [128, 2, 9, 128], MMDT, kind="ExternalInput").ap()
    cst_ext = nc.dram_tensor("cst", [128, 4], F32, kind="ExternalInput").ap()
    out_ext = nc.dram_tensor("out", [128, H, W], F32, kind="ExternalOutput").ap()

    with tile.TileContext(nc) as tc:
        with tc.tile_pool(name="sb", bufs=1) as sb, \
             tc.tile_pool(name="ps", bufs=8, space="PSUM") as ps, \
             tc.tile_pool(name="fin", bufs=8) as fin, \
             tc.tile_pool(name="fin4", bufs=4) as fin4, \
             tc.tile_pool(name="dram", bufs=1, space="DRAM") as dram:

            x_pad = sb.tile([128, HP * WP], MMDT, tag="x_pad")
            x_sw = sb.tile([128, HP * WP], MMDT, tag="x_sw")
            norm_pad = sb.tile([128, HP * WP], MMDT, tag="norm_pad")
            norm_sw = sb.tile([128, HP * WP], MMDT, tag="norm_sw")
            raw = sb.tile([128, H * W], BF16, tag="raw")
            wsb = sb.tile([128, 2 * 9 * 128], MMDT, tag="wsb")
            wssb = sb.tile([128, 2 * 9 * 128], MMDT, tag="wssb")
            cst = sb.tile([128, 4], F32, tag="cst")
            st6 = [sb.tile([128, NT * 6], F32, tag=f"st6_{c}", name=f"st6_{c}")
                   for c in range(2)]
            gst = sb.tile([128, 3 * 2], F32, tag="gst")
            params = sb.tile([128, 4], F32, tag="params")   # a1 b1 a2 b2
            sml = sb.tile([128, 16], F32, tag="sml")        # small scratch

            cc_in = dram.tile([128 * 3], F32)
            cc_out = dram.tile([128 * 3], F32)
            warm_in = dram.tile([128 * 3], F32)
            warm_out = dram.tile([128 * 3], F32)

            x3 = x_pad.rearrange("p (h w) -> p h w", h=HP)
            xs3 = x_sw.rearrange("p (h w) -> p h w", h=HP)
            n3 = norm_pad.rearrange("p (h w) -> p h w", h=HP)
            ns3 = norm_sw.rearrange("p (h w) -> p h w", h=HP)
            wv = wsb.rearrange("p (c t m) -> p c t m", c=2, t=9)
            wsv = wssb.rearrange("p (c t m) -> p c t m", c=2, t=9)

            # ---- phase 0: warmup collective doorbell FIRST on gpsimd ----
            # (same payload shape as the stats ARs so the one-time costs of
            # the collective path are paid here, hidden under conv1)
            nc.gpsimd.collective_compute(
                "AllReduce", ALU.add,
                replica_groups=[list(range(N_CORES))],
                ins=[warm_in.opt()], outs=[warm_out.opt()])

            # conv1 weights first (gate for the first matmul), conv2's later
            nc.scalar.dma_start(out=wsb[:, 0:9 * 128],
                                in_=w_ext.rearrange("k c t m -> k (c t m)")[:, 0:9 * 128])
            nc.scalar.dma_start(out=wssb[:, 0:9 * 128],
                                in_=ws_ext.rearrange("k c t m -> k (c t m)")[:, 0:9 * 128])
            nc.scalar.dma_start(out=cst[:, :], in_=cst_ext)

            # norm_pad/norm_sw borders <- zeros (conv2's padding) on DVE, off
            # the DMA queues entirely
            for img in (n3, ns3):
                nc.vector.memset(img[:, 0, :], 0.0)
                nc.vector.memset(img[:, HP - 1, :], 0.0)
                nc.vector.memset(img[:, 1:HP - 1, 0], 0.0)
                nc.vector.memset(img[:, 1:HP - 1, WP - 1], 0.0)

            # x (pre-padded on host) in chunks round-robin across queues;
            # small leading chunks so conv1 tile 0 is unblocked quickly
            # both image copies, first rows front-loaded; sync carries xp's
            # early rows, gpsimd+scalar carry the swapped copy's
            # early chunks (both copies) strictly alternate sync/gpsimd in
            # need-order; the scalar queue's head is busy with w1/ws1/cst, so
            # it only takes the late chunks (needed >25us in)
            bounds = [0, 6, 12, 18, 24, 30, 38, 48, 60, 74, 92, 110, HP]
            eng_x = [nc.sync, nc.gpsimd, nc.sync, nc.gpsimd,
                     nc.sync, nc.gpsimd, nc.sync, nc.gpsimd,
                     nc.scalar, nc.scalar, nc.scalar, nc.scalar]
            eng_xs = [nc.gpsimd, nc.sync, nc.gpsimd, nc.sync,
                      nc.gpsimd, nc.sync, nc.gpsimd, nc.sync,
                      nc.scalar, nc.scalar, nc.scalar, nc.scalar]
            for ch in range(len(bounds) - 1):
                r0, r1 = bounds[ch], bounds[ch + 1]
                eng_x[ch].dma_start(out=x3[:, r0:r1, :], in_=xp_ext[:, r0:r1, :])
                eng_xs[ch].dma_start(out=xs3[:, r0:r1, :], in_=xs_ext[:, r0:r1, :])
            # conv2 weights: needed only ~60us in, keep off the early queues
            nc.scalar.dma_start(out=wsb[:, 9 * 128:],
                                in_=w_ext.rearrange("k c t m -> k (c t m)")[:, 9 * 128:])
            nc.scalar.dma_start(out=wssb[:, 9 * 128:],
                                in_=ws_ext.rearrange("k c t m -> k (c t m)")[:, 9 * 128:])

            # ---- local stats -> AllReduce doorbell (all queue-local) ----
            def stats_allreduce(st6_t):
                s3 = sml[:, 4:7]
                mv = s3[:, 0:2]
                nc.vector.bn_aggr(mv, st6_t.rearrange("p (t k) -> p t k", k=6))
                nc.vector.tensor_mul(s3[:, 2:3], mv[:, 0:1], mv[:, 0:1])
                nc.gpsimd.dma_start(out=cc_in[:], in_=s3)
                nc.gpsimd.collective_compute(
                    "AllReduce", ALU.add,
                    replica_groups=[list(range(N_CORES))],
                    ins=[cc_in.opt()], outs=[cc_out.opt()])

            # ---- AllReduce result -> per-channel scale/bias ----
            def bn_params(gamma_ap, beta_ap, a_ap, b_ap):
                # bring back both sample-halves: dest [p, k, s], k=3 stats
                src = cc_out.rearrange("(s c k) -> c k s", s=2, k=3)
                nc.sync.dma_start(out=gst.rearrange("p (k s) -> p k s", k=3)[0:64],
                                  in_=src)
                nc.gpsimd.dma_start(out=gst.rearrange("p (k s) -> p k s", k=3)[64:128],
                                    in_=src)
                gsum = sml[:, 8:11]
                nc.vector.tensor_reduce(gsum, gst.rearrange("p (k s) -> p k s", k=3),
                                        axis=mybir.AxisListType.X, op=ALU.add)
                # gsum = [S0, S1, S2] sums over the 16 groups
                S0, S1, S2 = gsum[:, 0:1], gsum[:, 1:2], gsum[:, 2:3]
                r0 = sml[:, 11:12]
                nc.vector.tensor_add(r0, S1, S2)
                r1 = sml[:, 12:13]
                nc.vector.tensor_mul(r1, S0, S0)
                # ve = var + eps = r0/16 - r1/256 + eps
                r0e = sml[:, 13:14]
                nc.vector.tensor_scalar(r0e, r0, 1.0 / NGROUPS, BN_EPS,
                                        op0=ALU.mult, op1=ALU.add)
                ve = sml[:, 14:15]
                nc.vector.scalar_tensor_tensor(ve, r1, -1.0 / (NGROUPS * NGROUPS),
                                               r0e, op0=ALU.mult, op1=ALU.add)
                sd = sml[:, 15:16]
                nc.scalar.activation(sd, ve, AF.Sqrt)
                y0 = sml[:, 3:4]
                nc.vector.reciprocal(y0, sd)
                # one Newton step for rsqrt accuracy: y1 = y0*(1.5 - 0.5*ve*y0^2)
                tn = sml[:, 2:3]
                nc.vector.tensor_mul(tn, ve, y0)
                nc.vector.tensor_mul(tn, tn, y0)
                nc.vector.tensor_scalar(tn, tn, -0.5, 1.5, op0=ALU.mult, op1=ALU.add)
                nc.vector.tensor_mul(y0, y0, tn)
                nc.vector.tensor_mul(a_ap, y0, gamma_ap)
                # b = beta - a * S0/16
                nc.vector.tensor_mul(tn, S0, a_ap)
                nc.vector.scalar_tensor_tensor(b_ap, tn, -1.0 / NGROUPS, beta_ap,
                                               op0=ALU.mult, op1=ALU.add)

            # ---- conv1: 4-quadrant PE tiling (2 taps x 2 samples per round,
            # full 128x128 array useful). Even taps accumulate via quadrants
            # (0,0)/(64,64) into pb0[0:64]/pb1[64:128]; odd taps stream the
            # partition-swapped image/weights via (64,0)/(0,64) into
            # pb1[0:64]/pb0[64:128]. tile_position auto-derives from
            # base_partition of lhsT/out. raw = pb0 + pb1 on DVE.
            for t in range(NT):
                pb0 = ps.tile([128, N], F32, tag="psum", name=f"c1b0_{t}")
                pb1 = ps.tile([128, N], F32, tag="psum", name=f"c1b1_{t}")
                r0 = t * TR
                for r in range(5):
                    p = 2 * r
                    khp, kwp = p // 3, p % 3
                    rhp = slice(r0 + khp, r0 + khp + TR)
                    nc.tensor.matmul(pb0[0:64, :], wv[0:64, 0, p, 0:64],
                                     x3[0:64, rhp, kwp:kwp + W],
                                     start=(r == 0), stop=(r == 4))
                    nc.tensor.matmul(pb1[64:128, :], wv[64:128, 0, p, 64:128],
                                     x3[64:128, rhp, kwp:kwp + W],
                                     start=(r == 0), stop=(r == 4))
                    if r < 4:
                        q = 2 * r + 1
                        khq, kwq = q // 3, q % 3
                        rhq = slice(r0 + khq, r0 + khq + TR)
                        nc.tensor.matmul(pb1[0:64, :], wsv[64:128, 0, q, 0:64],
                                         xs3[64:128, rhq, kwq:kwq + W],
                                         start=(r == 0), stop=(r == 3))
                        nc.tensor.matmul(pb0[64:128, :], wsv[0:64, 0, q, 64:128],
                                         xs3[0:64, rhq, kwq:kwq + W],
                                         start=(r == 0), stop=(r == 3))
                # evacuate (one PSUM operand per engine op). ACT is idle in
                # conv1, so alternate: even tiles ACT copies one bank and DVE
                # adds from PSUM; odd tiles ACT copies BOTH banks and DVE does
                # a cheap bf16 SBUF add — balances ACT/DVE at ~1us/tile so the
                # phase stays PE-bound.
                rt = raw[:, t * N:(t + 1) * N]
                tmp = fin4.tile([128, N], BF16, tag="etmp", name=f"etmp_{t}")
                nc.scalar.activation(tmp[:, :], pb0[:, :], AF.Copy)
                if t % 2 == 0:
                    nc.vector.tensor_add(rt, pb1[:, :], tmp[:, :])
                else:
                    tmp1 = fin4.tile([128, N], BF16, tag="etmp",
                                     name=f"etmp1_{t}")
                    nc.scalar.activation(tmp1[:, :], pb1[:, :], AF.Copy)
                    nc.vector.tensor_add(rt, tmp[:, :], tmp1[:, :])
                nc.vector.bn_stats(st6[0][:, t * 6:(t + 1) * 6], rt)
            stats_allreduce(st6[0])

            # PE warm-keepers across the AllReduce stall (results unused);
            # shares the conv psum ring (same tag) to stay within 8 banks
            warm_ps = ps.tile([128, N], F32, tag="psum", name="warm_ps")
            for i in range(N_DUMMY):
                nc.tensor.matmul(warm_ps[:, :], wv[:, 0, i % 9, :],
                                 x3[:, (i % 16) * 8:(i % 16) * 8 + TR, 1:1 + W],
                                 start=True, stop=True)

            bn_params(cst[:, 0:1], cst[:, 1:2], params[:, 0:1], params[:, 1:2])

            # norm1: relu(a1*raw + b1) -> norm_pad interior. Interleaved with
            # conv2 emission (2 tiles ahead) so conv2's PSUM evacuations are
            # not queued behind the whole norm1 backlog on ACT's strict FIFO.
            def norm1_tile(t):
                rt = raw[:, t * N:(t + 1) * N].rearrange("p (a b) -> p a b", a=TR)
                dst = n3[:, 1 + t * TR:1 + (t + 1) * TR, 1:1 + W]
                nc.scalar.activation(dst, rt, AF.Relu,
                                     scale=params[:, 0:1], bias=params[:, 1:2])

            # swap_tile: partition-swapped copy of norm1's output rows for the
            # odd-tap quadrants (SBUF->SBUF DMA on the idle sync/gpsimd
            # queues; border columns come along, already zeroed)
            def swap_tile(t):
                rows = slice(1 + t * TR, 1 + (t + 1) * TR)
                nc.sync.dma_start(out=ns3[64:128, rows, :], in_=n3[0:64, rows, :])
                nc.gpsimd.dma_start(out=ns3[0:64, rows, :], in_=n3[64:128, rows, :])

            for t in range(4):
                norm1_tile(t)
                swap_tile(t)
            for t in range(NT):
                if t + 4 < NT:
                    norm1_tile(t + 4)
                    swap_tile(t + 4)
                pb0 = ps.tile([128, N], F32, tag="psum", name=f"c2b0_{t}")
                pb1 = ps.tile([128, N], F32, tag="psum", name=f"c2b1_{t}")
                r0 = t * TR
                for r in range(5):
                    p = 2 * r
                    khp, kwp = p // 3, p % 3
                    rhp = slice(r0 + khp, r0 + khp + TR)
                    nc.tensor.matmul(pb0[0:64, :], wv[0:64, 1, p, 0:64],
                                     n3[0:64, rhp, kwp:kwp + W],
                                     start=(r == 0), stop=(r == 4))
                    nc.tensor.matmul(pb1[64:128, :], wv[64:128, 1, p, 64:128],
                                     n3[64:128, rhp, kwp:kwp + W],
                                     start=(r == 0), stop=(r == 4))
                    if r < 4:
                        q = 2 * r + 1
                        khq, kwq = q // 3, q % 3
                        rhq = slice(r0 + khq, r0 + khq + TR)
                        nc.tensor.matmul(pb1[0:64, :], wsv[64:128, 1, q, 0:64],
                                         ns3[64:128, rhq, kwq:kwq + W],
                                         start=(r == 0), stop=(r == 3))
                        nc.tensor.matmul(pb0[64:128, :], wsv[0:64, 1, q, 64:128],
                                         ns3[0:64, rhq, kwq:kwq + W],
                                         start=(r == 0), stop=(r == 3))
                rt2 = raw[:, t * N:(t + 1) * N]
                tmp = fin4.tile([128, N], BF16, tag="etmp", name=f"etmp2_{t}")
                nc.scalar.activation(tmp[:, :], pb0[:, :], AF.Copy)
                nc.vector.tensor_add(rt2, pb1[:, :], tmp[:, :])
                nc.vector.bn_stats(st6[1][:, t * 6:(t + 1) * 6], rt2)
            stats_allreduce(st6[1])
            bn_params(cst[:, 2:3], cst[:, 3:4], params[:, 2:3], params[:, 3:4])

            # final: relu(a2*raw2 + b2 + x) -> DMA out
            # DVE computes s = a2*raw2 + x (bf16 stage, 2x DVE rate); the
            # (+b2, relu, ->fp32) step runs on ACT for most tiles but on DVE
            # (tensor_scalar add/max) for every 5th tile to balance the pipe.
            out_engines = [nc.sync, nc.gpsimd]
            for t in range(NT):
                rt = raw[:, t * N:(t + 1) * N].rearrange("p (a b) -> p a b", a=TR)
                xt = x3[:, 1 + t * TR:1 + (t + 1) * TR, 1:1 + W]
                if not USE_BF16_MM:
                    xt = xt.bitcast(F32)
                st = fin4.tile([128, TR, W], BF16, tag="fin_s")
                ft = fin.tile([128, TR, W], F32, tag="fin")
                nc.vector.scalar_tensor_tensor(st[:, :, :], rt, params[:, 2:3], xt,
                                               op0=ALU.mult, op1=ALU.add)
                if t % 5 == 4:
                    nc.vector.tensor_scalar(ft[:, :, :], st[:, :, :],
                                            params[:, 3:4], 0.0,
                                            op0=ALU.add, op1=ALU.max)
                else:
                    nc.scalar.activation(ft[:, :, :], st[:, :, :], AF.Relu,
                                         bias=params[:, 3:4])
                out_engines[t % 2].dma_start(out=out_ext[:, t * TR:(t + 1) * TR, :],
                                             in_=ft[:, :, :])

    nc.compile()
    return nc


def _get_nc():
    if "nc" not in _CACHE:
        _CACHE["nc"] = _build()
    return _CACHE["nc"]


def _pack_inputs(x, filters1, filters2, gamma1, beta1, gamma2, beta2):
    import ml_dtypes
    mmdt = ml_dtypes.bfloat16 if USE_BF16_MM else np.float32
    x = np.ascontiguousarray(x, dtype=np.float32)
    in_maps = []
    gb = np.stack([np.tile(np.asarray(g, np.float32), 2) for g in
                   (gamma1, beta1, gamma2, beta2)], axis=1)  # [128, 4]
    for i in range(N_CORES):
        s0, s1 = SPC * i, SPC * i + 1
        xp = np.zeros((128, HP, WP), mmdt)
        xp[0:C, 1:1 + H, 1:1 + W] = x[s0]
        xp[C:128, 1:1 + H, 1:1 + W] = x[s1]
        w = np.zeros((128, 2, 9, 128), mmdt)
        for ci, f in enumerate((filters1, filters2)):
            f = np.asarray(f, np.float32)
            # w[k, ci, tap, m]: lhsT[k=cin, m=cout], block-diagonal over samples
            fs0 = f[s0].transpose(1, 2, 3, 0).reshape(C, 9, C)   # [cin, tap, cout]
            fs1 = f[s1].transpose(1, 2, 3, 0).reshape(C, 9, C)
            w[0:C, ci, :, 0:C] = fs0
            w[C:128, ci, :, C:128] = fs1
        # partition-swapped copies for 4-quadrant convs
        xs = np.concatenate([xp[C:128], xp[0:C]], axis=0)
        ws = np.concatenate([w[C:128], w[0:C]], axis=0)         # [128, 2, 9, 128]
        in_maps.append({"xp": xp, "xs": xs, "w": w, "ws": ws, "cst": gb})
    return in_maps


def _run(in_maps, trace=False):
    nc = _get_nc()
    return run_bass_kernel_spmd(nc, in_maps, core_ids=list(range(N_CORES)),
                                trace=trace)


def kernel(x, filters1, filters2, gamma1, beta1, gamma2, beta2):
    in_maps = _pack_inputs(x, filters1, filters2, gamma1, beta1, gamma2, beta2)
    res = _run(in_maps, trace=False)
    out = np.empty((B, C, H, W), np.float32)
    for i in range(N_CORES):
        o = res.results[i]["out"]
        out[SPC * i] = o[0:C]
        out[SPC * i + 1] = o[C:128]
    return out


# revision 39
# speedup vs baseline: 1.0456x; 1.0456x over previous
"""Trainium2 Bass kernel for nn_BasicBlock (per-sample dynamic 3x3 convs +
sync-BN + residual ReLU), data-parallel over batch on 8 NeuronCores.

Reference semantics (B=16, C=64, H=W=128):
    out = relu(bn2(conv2(relu(bn1(conv1(x, f1))), f2)) + x)
with training-mode BN over full-batch (N,H,W) statistics.

Sharding: 2 samples per core. Per-sample convs become block-diagonal
128x128 matmuls (partitions 0-63 = sample A channels, 64-127 = sample B).
BN batch statistics are made exact via a tiny AllReduce of per-channel
(mean, var, mean^2) over the 16 (sample, core) groups.

conv1 runs as 4-quadrant PE-tiled matmuls (tile_position via
base_partition): per round, two taps x two samples stream concurrently
through the four 64x64 quadrant groups of the PE array (100% useful
weights), using host-provided partition-swapped copies of the padded
image and conv1 weights; even/odd tap chains accumulate in two PSUM
banks summed by DVE at evacuation (9 taps -> 5 rounds, 1.8x fewer
streaming cycles). conv2 stays block-diagonal 9-tap ([128,512] PSUM
accumulation). Matmul inputs are bf16; accumulation is fp32. Raw conv
outputs are staged bf16 for BN stats, normalization math is fp32.

v2 critical-path work:
- warmup AllReduce doorbell first-thing on gpsimd, sized like the real
  stats AR, so the one-time CC entry barrier + first-AR costs burn during
  conv1 instead of in front of the BN1 AllReduce.
- stats DMA + collective doorbell both on gpsimd (queue-local ordering,
  no cross-engine semaphore hop).
- norm_pad borders zeroed by DVE memsets, not element-strided DMAs
  (the old column DMAs emitted ~33k 2-byte packets).
- conv1 weights DMA'd separately ahead of conv2 weights; x chunks
  front-loaded so the first matmul issues as early as possible.
- dummy matmuls between conv1 and conv2 keep the PE HAM clock at 2.4GHz
  across the AllReduce stall (otherwise conv2 starts at 1.2GHz).
"""
import numpy as np

import concourse.bass as bass
import concourse.mybir as mybir
import concourse.tile as tile
from concourse import bacc
from concourse.bass_utils import run_bass_kernel_spmd
N_CORES = 8
B, C, H, W = 16, 64, 128, 128
SPC = B // N_CORES            # samples per core (2)
HP, WP = H + 2, W + 2         # padded image
TR = 4                        # image rows per spatial tile
NT = H // TR                  # 32 tiles
N = TR * W                    # 512 moving elements per matmul
NGROUPS = B                   # 16 (sample, core) stat groups of H*W each
BN_EPS = 1e-5
N_DUMMY = 110                 # PE warm-keeper matmuls during the BN1 AR stall

F32 = mybir.dt.float32
F32R = mybir.dt.float32r
BF16 = mybir.dt.bfloat16
USE_BF16_MM = True          # bf16 matmul inputs (fast weight load) vs f32r
MMDT = BF16 if USE_BF16_MM else F32R
AF = mybir.ActivationFunctionType
ALU = mybir.AluOpType

_CACHE = {}


def _build():
    nc = bacc.Bacc("TRN2", target_bir_lowering=False, debug=False,
                   num_devices=N_CORES)
    xp_ext = nc.dram_tensor("xp", [128, HP, WP], MMDT, kind="ExternalInput").ap()
    # partition-swapped copy of xp (sample B on 0:64, A on 64:128) for the
    # 4-quadrant conv1 packing
    xs_ext = nc.dram_tensor("xs", [128, HP, WP], MMDT, kind="ExternalInput").ap()
    w_ext = nc.dram_tensor("w", [128, 2, 9, 128], MMDT, kind="ExternalInput").ap()
    # partition-swapped weights, both convs (for quadrants fed by swapped images)
    ws_ext = nc.dram_tensor("ws", [128, 2, 9, 128], MMDT, kind="ExternalInput").ap()
    cst_ext = nc.dram_tensor("cst", [128, 4], F32, kind="ExternalInput").ap()
    out_ext = nc.dram_tensor("out", [128, H, W], F32, kind="ExternalOutput").ap()

    with tile.TileContext(nc) as tc:
        with tc.tile_pool(name="sb", bufs=1) as sb, \
             tc.tile_pool(name="ps", bufs=8, space="PSUM") as ps, \
             tc.tile_pool(name="fin", bufs=8) as fin, \
             tc.tile_pool(name="fin4", bufs=4) as fin4, \
             tc.tile_pool(name="dram", bufs=1, space="DRAM") as dram:

            x_pad = sb.tile([128, HP * WP], MMDT, tag="x_pad")
            x_sw = sb.tile([128, HP * WP], MMDT, tag="x_sw")
            norm_pad = sb.tile([128, HP * WP], MMDT, tag="norm_pad")
            norm_sw = sb.tile([128, HP * WP], MMDT, tag="norm_sw")
            raw = sb.tile([128, H * W], BF16, tag="raw")
            wsb = sb.tile([128, 2 * 9 * 128], MMDT, tag="wsb")
            wssb = sb.tile([128, 2 * 9 * 128], MMDT, tag="wssb")
            cst = sb.tile([128, 4], F32, tag="cst")
            st6 = [sb.tile([128, NT * 6], F32, tag=f"st6_{c}", name=f"st6_{c}")
                   for c in range(2)]
            gst = sb.tile([128, 3 * 2], F32, tag="gst")
            params = sb.tile([128, 4], F32, tag="params")   # a1 b1 a2 b2
            sml = sb.tile([128, 16], F32, tag="sml")        # small scratch

            cc_in = dram.tile([128 * 3], F32)
            cc_out = dram.tile([128 * 3], F32)
            warm_in = dram.tile([128 * 3], F32)
            warm_out = dram.tile([128 * 3], F32)

            x3 = x_pad.rearrange("p (h w) -> p h w", h=HP)
            xs3 = x_sw.rearrange("p (h w) -> p h w", h=HP)
            n3 = norm_pad.rearrange("p (h w) -> p h w", h=HP)
            ns3 = norm_sw.rearrange("p (h w) -> p h w", h=HP)
            wv = wsb.rearrange("p (c t m) -> p c t m", c=2, t=9)
            wsv = wssb.rearrange("p (c t m) -> p c t m", c=2, t=9)

            # ---- phase 0: warmup collective doorbell FIRST on gpsimd ----
            # (same payload shape as the stats ARs so the one-time costs of
            # the collective path are paid here, hidden under conv1)
            nc.gpsimd.collective_compute(
                "AllReduce", ALU.add,
                replica_groups=[list(range(N_CORES))],
                ins=[warm_in.opt()], outs=[warm_out.opt()])

            # conv1 weights first (gate for the first matmul), conv2's later
            nc.scalar.dma_start(out=wsb[:, 0:9 * 128],
                                in_=w_ext.rearrange("k c t m -> k (c t m)")[:, 0:9 * 128])
            nc.scalar.dma_start(out=wssb[:, 0:9 * 128],
                                in_=ws_ext.rearrange("k c t m -> k (c t m)")[:, 0:9 * 128])
            nc.scalar.dma_start(out=cst[:, :], in_=cst_ext)

            # norm_pad/norm_sw borders <- zeros (conv2's padding) on DVE, off
            # the DMA queues entirely
            for img in (n3, ns3):
                nc.vector.memset(img[:, 0, :], 0.0)
                nc.vector.memset(img[:, HP - 1, :], 0.0)
                nc.vector.memset(img[:, 1:HP - 1, 0], 0.0)
                nc.vector.memset(img[:, 1:HP - 1, WP - 1], 0.0)

            # x (pre-padded on host) in chunks round-robin across queues;
            # small leading chunks so conv1 tile 0 is unblocked quickly
            # both image copies, first rows front-loaded; sync carries xp's
            # early rows, gpsimd+scalar carry the swapped copy's
            # few, larger chunks (queues drain ~85GB/s each and every
            # dma_start costs ~0.7us of issue overhead — many small chunks
            # starved conv1 tiles 2-4). Both copies' rows interleave strictly
            # across sync+gpsimd; weights stay alone on scalar.
            bounds = [0, 8, 20, 36, 56, 80, 104, HP]
            for ch in range(len(bounds) - 1):
                r0, r1 = bounds[ch], bounds[ch + 1]
                ea, eb = (nc.sync, nc.gpsimd) if ch % 2 == 0 else \
                         (nc.gpsimd, nc.sync)
                ea.dma_start(out=x3[:, r0:r1, :], in_=xp_ext[:, r0:r1, :])
                eb.dma_start(out=xs3[:, r0:r1, :], in_=xs_ext[:, r0:r1, :])
            # conv2 weights: needed only ~60us in, keep off the early queues
            nc.scalar.dma_start(out=wsb[:, 9 * 128:],
                                in_=w_ext.rearrange("k c t m -> k (c t m)")[:, 9 * 128:])
            nc.scalar.dma_start(out=wssb[:, 9 * 128:],
                                in_=ws_ext.rearrange("k c t m -> k (c t m)")[:, 9 * 128:])

            # ---- local stats -> AllReduce doorbell (all queue-local) ----
            def stats_allreduce(st6_t):
                s3 = sml[:, 4:7]
                mv = s3[:, 0:2]
                nc.vector.bn_aggr(mv, st6_t.rearrange("p (t k) -> p t k", k=6))
                nc.vector.tensor_mul(s3[:, 2:3], mv[:, 0:1], mv[:, 0:1])
                nc.gpsimd.dma_start(out=cc_in[:], in_=s3)
                nc.gpsimd.collective_compute(
                    "AllReduce", ALU.add,
                    replica_groups=[list(range(N_CORES))],
                    ins=[cc_in.opt()], outs=[cc_out.opt()])

            # ---- AllReduce result -> per-channel scale/bias ----
            def bn_params(gamma_ap, beta_ap, a_ap, b_ap):
                # bring back both sample-halves: dest [p, k, s], k=3 stats
                src = cc_out.rearrange("(s c k) -> c k s", s=2, k=3)
                nc.sync.dma_start(out=gst.rearrange("p (k s) -> p k s", k=3)[0:64],
                                  in_=src)
                nc.gpsimd.dma_start(out=gst.rearrange("p (k s) -> p k s", k=3)[64:128],
                                    in_=src)
                gsum = sml[:, 8:11]
                nc.vector.tensor_reduce(gsum, gst.rearrange("p (k s) -> p k s", k=3),
                                        axis=mybir.AxisListType.X, op=ALU.add)
                # gsum = [S0, S1, S2] sums over the 16 groups
                S0, S1, S2 = gsum[:, 0:1], gsum[:, 1:2], gsum[:, 2:3]
                r0 = sml[:, 11:12]
                nc.vector.tensor_add(r0, S1, S2)
                r1 = sml[:, 12:13]
                nc.vector.tensor_mul(r1, S0, S0)
                # ve = var + eps = r0/16 - r1/256 + eps
                r0e = sml[:, 13:14]
                nc.vector.tensor_scalar(r0e, r0, 1.0 / NGROUPS, BN_EPS,
                                        op0=ALU.mult, op1=ALU.add)
                ve = sml[:, 14:15]
                nc.vector.scalar_tensor_tensor(ve, r1, -1.0 / (NGROUPS * NGROUPS),
                                               r0e, op0=ALU.mult, op1=ALU.add)
                sd = sml[:, 15:16]
                nc.scalar.activation(sd, ve, AF.Sqrt)
                y0 = sml[:, 3:4]
                nc.vector.reciprocal(y0, sd)
                # one Newton step for rsqrt accuracy: y1 = y0*(1.5 - 0.5*ve*y0^2)
                tn = sml[:, 2:3]
                nc.vector.tensor_mul(tn, ve, y0)
                nc.vector.tensor_mul(tn, tn, y0)
                nc.vector.tensor_scalar(tn, tn, -0.5, 1.5, op0=ALU.mult, op1=ALU.add)
                nc.vector.tensor_mul(y0, y0, tn)
                nc.vector.tensor_mul(a_ap, y0, gamma_ap)
                # b = beta - a * S0/16
                nc.vector.tensor_mul(tn, S0, a_ap)
                nc.vector.scalar_tensor_tensor(b_ap, tn, -1.0 / NGROUPS, beta_ap,
                                               op0=ALU.mult, op1=ALU.add)

            # ---- conv1: 4-quadrant PE tiling (2 taps x 2 samples per round,
            # full 128x128 array useful). Even taps accumulate via quadrants
            # (0,0)/(64,64) into pb0[0:64]/pb1[64:128]; odd taps stream the
            # partition-swapped image/weights via (64,0)/(0,64) into
            # pb1[0:64]/pb0[64:128]. tile_position auto-derives from
            # base_partition of lhsT/out. raw = pb0 + pb1 on DVE.
            for t in range(NT):
                pb0 = ps.tile([128, N], F32, tag="psum", name=f"c1b0_{t}")
                pb1 = ps.tile([128, N], F32, tag="psum", name=f"c1b1_{t}")
                r0 = t * TR
                for r in range(5):
                    p = 2 * r
                    khp, kwp = p // 3, p % 3
                    rhp = slice(r0 + khp, r0 + khp + TR)
                    nc.tensor.matmul(pb0[0:64, :], wv[0:64, 0, p, 0:64],
                                     x3[0:64, rhp, kwp:kwp + W],
                                     start=(r == 0), stop=(r == 4))
                    nc.tensor.matmul(pb1[64:128, :], wv[64:128, 0, p, 64:128],
                                     x3[64:128, rhp, kwp:kwp + W],
                                     start=(r == 0), stop=(r == 4))
                    if r < 4:
                        q = 2 * r + 1
                        khq, kwq = q // 3, q % 3
                        rhq = slice(r0 + khq, r0 + khq + TR)
                        nc.tensor.matmul(pb1[0:64, :], wsv[64:128, 0, q, 0:64],
                                         xs3[64:128, rhq, kwq:kwq + W],
                                         start=(r == 0), stop=(r == 3))
                        nc.tensor.matmul(pb0[64:128, :], wsv[0:64, 0, q, 64:128],
                                         xs3[0:64, rhq, kwq:kwq + W],
                                         start=(r == 0), stop=(r == 3))
                # evacuate (one PSUM operand per engine op). ACT is idle in
                # conv1, so alternate: even tiles ACT copies one bank and DVE
                # adds from PSUM; odd tiles ACT copies BOTH banks and DVE does
                # a cheap bf16 SBUF add — balances ACT/DVE at ~1us/tile so the
                # phase stays PE-bound.
                rt = raw[:, t * N:(t + 1) * N]
                tmp = fin4.tile([128, N], BF16, tag="etmp", name=f"etmp_{t}")
                nc.scalar.activation(tmp[:, :], pb0[:, :], AF.Copy)
                if t % 2 == 0:
                    nc.vector.tensor_add(rt, pb1[:, :], tmp[:, :])
                else:
                    tmp1 = fin4.tile([128, N], BF16, tag="etmp",
                                     name=f"etmp1_{t}")
                    nc.scalar.activation(tmp1[:, :], pb1[:, :], AF.Copy)
                    nc.vector.tensor_add(rt, tmp[:, :], tmp1[:, :])
                nc.vector.bn_stats(st6[0][:, t * 6:(t + 1) * 6], rt)
            stats_allreduce(st6[0])

            # PE warm-keepers across the AllReduce stall (results unused);
            # shares the conv psum ring (same tag) to stay within 8 banks
            warm_ps = ps.tile([128, N], F32, tag="psum", name="warm_ps")
            for i in range(N_DUMMY):
                nc.tensor.matmul(warm_ps[:, :], wv[:, 0, i % 9, :],
                                 x3[:, (i % 16) * 8:(i % 16) * 8 + TR, 1:1 + W],
                                 start=True, stop=True)

            bn_params(cst[:, 0:1], cst[:, 1:2], params[:, 0:1], params[:, 1:2])

            # norm1: relu(a1*raw + b1) -> norm_pad interior. Interleaved with
            # conv2 emission (2 tiles ahead) so conv2's PSUM evacuations are
            # not queued behind the whole norm1 backlog on ACT's strict FIFO.
            def norm1_tile(t):
                rt = raw[:, t * N:(t + 1) * N].rearrange("p (a b) -> p a b", a=TR)
                dst = n3[:, 1 + t * TR:1 + (t + 1) * TR, 1:1 + W]
                nc.scalar.activation(dst, rt, AF.Relu,
                                     scale=params[:, 0:1], bias=params[:, 1:2])

            # swap_tile: partition-swapped copy of norm1's output rows for the
            # odd-tap quadrants (SBUF->SBUF DMA on the idle sync/gpsimd
            # queues; border columns come along, already zeroed)
            def swap_tile(t):
                rows = slice(1 + t * TR, 1 + (t + 1) * TR)
                nc.sync.dma_start(out=ns3[64:128, rows, :], in_=n3[0:64, rows, :])
                nc.gpsimd.dma_start(out=ns3[0:64, rows, :], in_=n3[64:128, rows, :])

            for t in range(4):
                norm1_tile(t)
                swap_tile(t)
            for t in range(NT):
                if t + 4 < NT:
                    norm1_tile(t + 4)
                    swap_tile(t + 4)
                pb0 = ps.tile([128, N], F32, tag="psum", name=f"c2b0_{t}")
                pb1 = ps.tile([128, N], F32, tag="psum", name=f"c2b1_{t}")
                r0 = t * TR
                for r in range(5):
                    p = 2 * r
                    khp, kwp = p // 3, p % 3
                    rhp = slice(r0 + khp, r0 + khp + TR)
                    nc.tensor.matmul(pb0[0:64, :], wv[0:64, 1, p, 0:64],
                                     n3[0:64, rhp, kwp:kwp + W],
                                     start=(r == 0), stop=(r == 4))
                    nc.tensor.matmul(pb1[64:128, :], wv[64:128, 1, p, 64:128],
                                     n3[64:128, rhp, kwp:kwp + W],
                                     start=(r == 0), stop=(r == 4))
                    if r < 4:
                        q = 2 * r + 1
                        khq, kwq = q // 3, q % 3
                        rhq = slice(r0 + khq, r0 + khq + TR)
                        nc.tensor.matmul(pb1[0:64, :], wsv[64:128, 1, q, 0:64],
                                         ns3[64:128, rhq, kwq:kwq + W],
                                         start=(r == 0), stop=(r == 3))
                        nc.tensor.matmul(pb0[64:128, :], wsv[0:64, 1, q, 64:128],
                                         ns3[0:64, rhq, kwq:kwq + W],
                                         start=(r == 0), stop=(r == 3))
                rt2 = raw[:, t * N:(t + 1) * N]
                tmp = fin4.tile([128, N], BF16, tag="etmp", name=f"etmp2_{t}")
                nc.scalar.activation(tmp[:, :], pb0[:, :], AF.Copy)
                nc.vector.tensor_add(rt2, pb1[:, :], tmp[:, :])
                nc.vector.bn_stats(st6[1][:, t * 6:(t + 1) * 6], rt2)
            stats_allreduce(st6[1])
            bn_params(cst[:, 2:3], cst[:, 3:4], params[:, 2:3], params[:, 3:4])

            # final: relu(a2*raw2 + b2 + x) -> DMA out
            # DVE computes s = a2*raw2 + x (bf16 stage, 2x DVE rate); the
            # (+b2, relu, ->fp32) step runs on ACT for most tiles but on DVE
            # (tensor_scalar add/max) for every 5th tile to balance the pipe.
            out_engines = [nc.sync, nc.gpsimd]
            for t in range(NT):
                rt = raw[:, t * N:(t + 1) * N].rearrange("p (a b) -> p a b", a=TR)
                xt = x3[:, 1 + t * TR:1 + (t + 1) * TR, 1:1 + W]
                if not USE_BF16_MM:
                    xt = xt.bitcast(F32)
                st = fin4.tile([128, TR, W], BF16, tag="fin_s")
                ft = fin.tile([128, TR, W], F32, tag="fin")
                nc.vector.scalar_tensor_tensor(st[:, :, :], rt, params[:, 2:3], xt,
                                               op0=ALU.mult, op1=ALU.add)
                if t % 5 == 4:
                    nc.vector.tensor_scalar(ft[:, :, :], st[:, :, :],
                                            params[:, 3:4], 0.0,
                                            op0=ALU.add, op1=ALU.max)
                else:
                    nc.scalar.activation(ft[:, :, :], st[:, :, :], AF.Relu,
                                         bias=params[:, 3:4])
                out_engines[t % 2].dma_start(out=out_ext[:, t * TR:(t + 1) * TR, :],
                                             in_=ft[:, :, :])

    nc.compile()
    return nc


def _get_nc():
    if "nc" not in _CACHE:
        _CACHE["nc"] = _build()
    return _CACHE["nc"]


def _pack_inputs(x, filters1, filters2, gamma1, beta1, gamma2, beta2):
    import ml_dtypes
    mmdt = ml_dtypes.bfloat16 if USE_BF16_MM else np.float32
    x = np.ascontiguousarray(x, dtype=np.float32)
    in_maps = []
    gb = np.stack([np.tile(np.asarray(g, np.float32), 2) for g in
                   (gamma1, beta1, gamma2, beta2)], axis=1)  # [128, 4]
    for i in range(N_CORES):
        s0, s1 = SPC * i, SPC * i + 1
        xp = np.zeros((128, HP, WP), mmdt)
        xp[0:C, 1:1 + H, 1:1 + W] = x[s0]
        xp[C:128, 1:1 + H, 1:1 + W] = x[s1]
        w = np.zeros((128, 2, 9, 128), mmdt)
        for ci, f in enumerate((filters1, filters2)):
            f = np.asarray(f, np.float32)
            # w[k, ci, tap, m]: lhsT[k=cin, m=cout], block-diagonal over samples
            fs0 = f[s0].transpose(1, 2, 3, 0).reshape(C, 9, C)   # [cin, tap, cout]
            fs1 = f[s1].transpose(1, 2, 3, 0).reshape(C, 9, C)
            w[0:C, ci, :, 0:C] = fs0
            w[C:128, ci, :, C:128] = fs1
        # partition-swapped copies for 4-quadrant convs
        xs = np.concatenate([xp[C:128], xp[0:C]], axis=0)
        ws = np.concatenate([w[C:128], w[0:C]], axis=0)         # [128, 2, 9, 128]
        in_maps.append({"xp": xp, "xs": xs, "w": w, "ws": ws, "cst": gb})
    return in_maps


def _run(in_maps, trace=False):
    nc = _get_nc()
    return run_bass_kernel_spmd(nc, in_maps, core_ids=list(range(N_CORES)),
                                trace=trace)


def kernel(x, filters1, filters2, gamma1, beta1, gamma2, beta2):
    in_maps = _pack_inputs(x, filters1, filters2, gamma1, beta1, gamma2, beta2)
    res = _run(in_maps, trace=False)
    out = np.empty((B, C, H, W), np.float32)
    for i in range(N_CORES):
        o = res.results[i]["out"]
        out[SPC * i] = o[0:C]
        out[SPC * i + 1] = o[C:128]
    return out
